# revision 23
# baseline (speedup 1.0000x reference)
# kernel.py — Trainium2 Bass kernel for nn_BubblePredictor (LSTM scan + per-step head)
#
# Math (per time step, PyTorch LSTMCell, bias=False):
#   gates = x_t @ W_ih.T + h @ W_hh.T          # [B, 4H], gate order (i, f, g, o)
#   c' = sigmoid(f) * c + sigmoid(i) * tanh(g)
#   h' = sigmoid(o) * tanh(c')
#   logit_t = h' @ W_out.T + b_out             # [B, 2]
# Returns (logits [B,T,2], h_T [B,H], c_T [B,H]).
#
# Strategy: data-parallel over batch (64 rows/core on 8 cores). Per core the
# scan runs on-chip with everything SBUF-resident:
#  - "Layout B": gate dim on partitions, batch on the free dim. Stationary
#    matmul operand = weights, moving operand = state. The per-step gates
#    matmul is out[128, b] += WT_k[:, m].T @ hhat_k where hhat = [h; x], K
#    chunks (128, 128, 8).
#  - batch split into 2 half-chunks (32 each) so the scheduler can overlap
#    PE (matmul) of one chunk with ACT/DVE (activations/state update) of the
#    other.
#  - All four gate activations are a single Sigmoid op: tanh(g) = 2*sig(2g)-1
#    with the 2x folded into W's g-rows; the affine fixup folds into the DVE
#    state update (scalar_tensor_tensor). tanh(c) = 2*sig(2c)-1 via the
#    activation's free scale. We store h' = h/2 (so h' = (sig(2c)-0.5)*o is a
#    single fused DVE op) and compensate by scaling W_hh's and W_out's
#    h-input columns by 2.
#  - Per-step logits head is batched: h history is kept in SBUF and the tiny
#    [*,2] head matmul runs as a tail (M=128 blocks of (t,b), N=2).
#  - b_out is added on the host (free), as is all layout unscrambling.

import os
import numpy as np
import ml_dtypes

import concourse.bass as bass
import concourse.bacc as bacc
import concourse.mybir as mybir
import concourse.tile as tile
from concourse.bass_utils import run_bass_kernel_spmd

B, T, D, H = 512, 512, 8, 256
NCORES = 8
BS = B // NCORES          # 64 batch rows per core (8 batch shards)
NCH = 2                   # sub-chunks per core (pipelining)
CB = BS // NCH            # 32
G4 = 4 * H                # 1024 gate rows
NM = G4 // 128            # 8 M-tiles
F32 = mybir.dt.float32

# matmul input dtype: "fp16" (fast, ~1e-3 err), "bf16" (fast, more err),
# "fp32" (exact, 4x slower PE)
MM_KIND = os.environ.get("BUBBLE_MM_DT", "fp16")
_DT_MAP = {
    "fp16": (mybir.dt.float16, np.float16),
    "bf16": (mybir.dt.bfloat16, ml_dtypes.bfloat16),
    "fp32": (mybir.dt.float32, np.float32),
}
MM_DT, NP_MM = _DT_MAP[MM_KIND]

_CACHE = {}


def build_program(t_steps=T, repeat=1, ablate=()):
    nc = bacc.Bacc(
        "TRN2",
        target_bir_lowering=False,
        debug=False,
        enable_asserts=False,
        num_devices=NCORES,
    )

    xT = nc.dram_tensor("xT", [D, t_steps // 2, BS, 2], MM_DT, kind="ExternalInput")
    w01 = nc.dram_tensor("w01", [2, 128, G4], MM_DT, kind="ExternalInput")
    w2d = nc.dram_tensor("w2d", [D, G4], MM_DT, kind="ExternalInput")
    woT = nc.dram_tensor("woT", [128, 2, 2], MM_DT, kind="ExternalInput")

    nbl = t_steps * BS // 128          # logits blocks of 128 (t,b) rows
    nhalf = max(1, nbl // 128)         # 128-block groups (PSUM bank sized)
    per_half = nbl // nhalf
    lgo = nc.dram_tensor("lgo", [nhalf, 128, per_half, 2], F32, kind="ExternalOutput")
    hTo = nc.dram_tensor("hTo", [128, 2, BS], MM_DT, kind="ExternalOutput")
    cTo = nc.dram_tensor("cTo", [128, NCH, 2, CB], F32, kind="ExternalOutput")

    with tile.TileContext(nc) as tc:
        with (
            tc.tile_pool(name="persist", bufs=1) as persist,
            tc.tile_pool(name="work", bufs=2) as work,
            tc.tile_pool(name="psum", bufs=2, space="PSUM") as psum,
            tc.tile_pool(name="lpsum", bufs=2, space="PSUM") as lpsum,
        ):
            xs = persist.tile([D, t_steps // 2, BS, 2], MM_DT)
            w0 = persist.tile([128, G4], MM_DT)
            w1 = persist.tile([128, G4], MM_DT)
            w2 = persist.tile([D, G4], MM_DT)
            wo = persist.tile([128, 2, 2], MM_DT)
            tw = t_steps // 2
            hb0 = persist.tile([128, 2, tw, BS], MM_DT)
            hb1 = persist.tile([128, 2, tw, BS], MM_DT)
            hbw = [hb0, hb1]
            cst = persist.tile([128, NCH, 2, CB], F32)

            if "state" in ablate:
                # keep read-only tensors "written" for Tile's release check
                nc.gpsimd.memset(cst[:], 0.0)
                nc.gpsimd.memset(hb0[:], 0.0)
                nc.gpsimd.memset(hb1[:], 0.0)
            nc.sync.dma_start(xs[:], xT[:])
            nc.sync.dma_start(w0[:], w01[0])
            nc.sync.dma_start(w1[:], w01[1])
            nc.sync.dma_start(w2[:], w2d[:])
            nc.sync.dma_start(wo[:], woT[:])

            wks = [w0, w1, w2]

            # PSUM bank per (chunk, step-pair): [m-tile(8), b(32), t2(2)],
            # 2 KB = exactly one bank. The 8 x-projection matmuls for a pair
            # of steps land once (N=64, start=True); each step's h-matmuls
            # then accumulate into the stride-2 t2 slices.
            def x_mms(ch, tp):
                cs = slice(ch * CB, (ch + 1) * CB)
                gps = psum.tile([128, NM, CB, 2], F32, tag=f"gps{ch}", name=f"gps{ch}")
                # start=True clears has_written for the WHOLE bank, so only
                # the first x-matmul may carry it; later writes to untouched
                # regions use start=False (plain write, sets has_written).
                for m in range(NM):
                    nc.tensor.matmul(
                        gps[:, m, :, :], w2[:, m * 128:(m + 1) * 128],
                        xs[:, tp, cs, :],
                        start=(m == 0), stop=False, skip_group_check=True)
                return gps

            def step_mm_sigma(ch, t, gps):
                cs = slice(ch * CB, (ch + 1) * CB)
                t2 = t % 2
                if t > 0 and "hmm" not in ablate:
                    hbp = hbw[(t - 1) // tw]
                    tp = (t - 1) % tw
                    for m in range(NM):
                        for k in (0, 1):
                            nc.tensor.matmul(
                                gps[:, m, :, t2], wks[k][:, m * 128:(m + 1) * 128],
                                hbp[:, k, tp, cs],
                                start=False, stop=(k == 1), skip_group_check=True)
                # all-gate sigmoid (g rows were pre-scaled 2x => sig holds
                # sig(2g) in the g slice); fp16 out enables DVE 2x mode
                sg = work.tile([128, NM, CB], F32, tag=f"sg{ch}", name=f"sg{ch}")
                if "sigma" not in ablate:
                    nc.scalar.activation(sg[:], gps[:, :, :, t2],
                                         mybir.ActivationFunctionType.Sigmoid)
                return sg

            def step_state(ch, t, sg):
                if "state" in ablate:
                    return
                cs = slice(ch * CB, (ch + 1) * CB)
                s_i = sg[:, 0:2, :]
                s_f = sg[:, 2:4, :]
                s_o = sg[:, 4:6, :]
                s_g = sg[:, 6:8, :]
                cv = cst[:, ch, :, :]
                # u = (sig(2g) - 0.5) * i   == tanh(g)/2 * i    (fp16, DVE 2x)
                u = work.tile([128, 2, CB], F32, tag=f"u{ch}", name=f"u{ch}")
                nc.vector.scalar_tensor_tensor(
                    u[:], s_g, -0.5, s_i,
                    op0=mybir.AluOpType.add, op1=mybir.AluOpType.mult)
                if t == 0:
                    # c = 2u
                    nc.vector.tensor_scalar_mul(cv, u[:], 2.0)
                else:
                    # v = f * c on the (otherwise idle) GPSIMD engine
                    v = work.tile([128, 2, CB], F32, tag=f"v{ch}", name=f"v{ch}")
                    nc.gpsimd.tensor_mul(v[:], s_f, cv)
                    # c = 2u + v
                    nc.vector.scalar_tensor_tensor(
                        cv, u[:], 2.0, v[:],
                        op0=mybir.AluOpType.mult, op1=mybir.AluOpType.add)
                # tc = tanh(c);  h' = tc * 0.5 * o  == h/2
                # (tanh avoids the (sig-0.5) cancellation under fp16 storage)
                sc = work.tile([128, 2, CB], MM_DT, tag=f"sc{ch}", name=f"sc{ch}")
                nc.scalar.activation(
                    sc[:], cv, mybir.ActivationFunctionType.Tanh)
                hw_ = hbw[t // tw]
                nc.vector.scalar_tensor_tensor(
                    hw_[:, :, t % tw, cs], sc[:], 0.5, s_o,
                    op0=mybir.AluOpType.mult, op1=mybir.AluOpType.mult)

            def whole_scan(_iv=None):
                for tp in range(t_steps // 2):
                    g_ab = [x_mms(ch, tp) for ch in range(NCH)]
                    for t2 in range(2):
                        t = 2 * tp + t2
                        # emit both chunks' matmuls+sigma first so the ACT
                        # queue runs [sigA, sigB, scA, scB] — B's gate sigmoid
                        # fills the gap while A's DVE chain computes c.
                        sgs = [step_mm_sigma(ch, t, g_ab[ch]) for ch in range(NCH)]
                        for ch in range(NCH):
                            step_state(ch, t, sgs[ch])

            if repeat == 1:
                whole_scan()
            else:
                tc.For_i_unrolled(0, repeat, 1, whole_scan, max_unroll=1)

            # ---- final state outputs (h' raw fp16; host rescales) ----
            nc.sync.dma_start(hTo[:], hb1[:, :, tw - 1, :])
            nc.sync.dma_start(cTo[:], cst[:])

            # ---- logits head tail: out[(t,b) block of 128, j] ----
            for half in range(nhalf if "logits" not in ablate else 0):
                lp = lpsum.tile([128, per_half, 2], F32, tag="lp", name="lp")
                for bi in range(per_half):
                    blk = half * per_half + bi
                    t0 = blk * 128 // BS
                    nt = 128 // BS
                    for hh in range(2):
                        nc.tensor.matmul(
                            lp[:, bi, :],
                            hbw[t0 // tw][:, hh, t0 % tw:t0 % tw + nt, :],
                            wo[:, hh, :],
                            start=(hh == 0), stop=(hh == 1))
                ls = work.tile([128, per_half, 2], F32, tag="ls", name="ls", bufs=1)
                nc.vector.tensor_copy(ls[:], lp[:])
                nc.sync.dma_start(lgo[half], ls[:])

    nc.compile()
    return nc


def prep_weights(W_ih, W_hh, W_out):
    # gate blocks in reference order: i [0:256) f [256:512) g [512:768) o [768:1024)
    # M-tile order (i0,i1,f0,f1,o0,o1,g0,g1)
    perm = np.concatenate([
        np.arange(0, 256), np.arange(256, 512),
        np.arange(768, 1024), np.arange(512, 768)])
    Wfull = np.concatenate([2.0 * W_hh, W_ih], axis=1)[perm]   # [1024, 264]
    Wfull[768:1024] *= 2.0          # g rows: tanh(g) = 2 sig(2g) - 1
    WT = np.ascontiguousarray(Wfull.T)                         # [264, 1024]
    w01 = np.stack([WT[0:128], WT[128:256]]).astype(NP_MM)
    w2 = np.ascontiguousarray(WT[256:264]).astype(NP_MM)
    woT = np.ascontiguousarray(
        (2.0 * W_out).T.reshape(2, 128, 2).transpose(1, 0, 2)).astype(NP_MM)
    return w01, w2, woT


def kernel(history, W_ih, W_hh, W_out, b_out):
    history = np.asarray(history, dtype=np.float32)
    W_ih = np.asarray(W_ih, dtype=np.float32)
    W_hh = np.asarray(W_hh, dtype=np.float32)
    W_out = np.asarray(W_out, dtype=np.float32)
    b_out = np.asarray(b_out, dtype=np.float32)

    t_steps = history.shape[1]
    b_full = history.shape[0]
    bs = b_full // NCORES
    assert bs == BS and t_steps % 2 == 0

    if t_steps not in _CACHE:
        _CACHE[t_steps] = build_program(t_steps)
    nc = _CACHE[t_steps]

    w01, w2, woT = prep_weights(W_ih, W_hh, W_out)
    in_maps = []
    for c in range(NCORES):
        hc = history[c * BS:(c + 1) * BS]                      # [BS, T, D]
        xT = np.ascontiguousarray(
            hc.transpose(2, 1, 0).reshape(D, t_steps // 2, 2, BS)
            .transpose(0, 1, 3, 2)).astype(NP_MM)
        in_maps.append({"xT": xT, "w01": w01, "w2d": w2, "woT": woT})

    res = run_bass_kernel_spmd(nc, in_maps, core_ids=list(range(NCORES)),
                               trace=bool(int(os.environ.get("BUBBLE_TRACE", "0"))))
    _CACHE["last_result"] = res
    logits = np.empty((b_full, t_steps, 2), np.float32)
    h_T = np.empty((b_full, H), np.float32)
    c_T = np.empty((b_full, H), np.float32)
    for c in range(NCORES):
        r = res.results[c]
        bsl = slice(c * BS, (c + 1) * BS)
        lgo, hTo, cTo = r["lgo"], r["hTo"], r["cTo"]
        # lgo [half, p, bi, j]; blk = half*per_half+bi covers (t,b) rows
        # blk*128 + p with t = row // BS, b = row % BS
        nhalf, _, per_half, _ = lgo.shape
        lg = lgo.transpose(0, 2, 1, 3).reshape(nhalf * per_half * 128, 2)
        lg = lg.reshape(t_steps, BS, 2).transpose(1, 0, 2)     # [BS, T, 2]
        logits[bsl] = lg + b_out
        h_T[bsl] = 2.0 * hTo.astype(np.float32).transpose(2, 1, 0).reshape(BS, H)
        c_T[bsl] = cTo.transpose(1, 3, 2, 0).reshape(BS, H)
    return logits, h_T, c_T
